# revision 1
# baseline (speedup 1.0000x reference)
"""CANLayer (two-edge-set multi-head cell attention + skip) on 8 TRN2 NeuronCores.

Self-contained: hardcodes shapes for N=50000 cells, E=800000 edges/set,
C_IN=128, HEADS=4, D_OUT=32.

Strategy:
 - Cells are 1D-partitioned across 8 cores (6272 aligned cells each); edges are
   routed to the core owning their target cell (host-side, part of sharding).
 - Each core redundantly computes per-node tables in DRAM:
     table[s][n] = [xm_s(n) as 128 bf16 | ss_s(n) as 4 f32 | pad]  (512B rows)
   where xm = x @ W_s and ss = x @ (W_s @ a_src_s) (attention source logit).
 - Edge phase: per 128-target-cell window, dma_gather pulls the 512B rows for
   each edge (int16 indices, split over two table halves); attention weights
   use the shift-free identity  softmax(LR(ss+sd)) == normalize over segment of
   exp(LR(ss+sd)), computed per edge with sd broadcast from the window's
   target cells via a one-hot^T matmul; aggregation is a one-hot matmul
   accumulated in PSUM (cells x [128 msg | 4 denom]).
 - Output: relu(agg_low/denom_low + agg_up/denom_up + EPS*(x@W_skip+b_skip)).
"""
import sys
sys.path.insert(0, "/opt/trn_rl_repo")

import os

import numpy as np
import ml_dtypes

import concourse.bass as bass
import concourse.mybir as mybir
import concourse.tile as tile
from concourse import bacc
from concourse.bass_utils import run_bass_kernel_spmd

BF16 = mybir.dt.bfloat16
F32 = mybir.dt.float32
I16 = mybir.dt.int16

N_CELLS = 50000
N_EDGES = 800000
C_IN = 128
HEADS = 4
D_OUT = 32
HD = HEADS * D_OUT          # 128
EPS = 1.0 + 1e-6
NEG_SLOPE = 0.01

N_CORES = 8
CPC = 6272                  # cells per core (49 * 128), last core ragged
NW = 49                     # windows (128 cells) per core
NT = 391                    # node tiles over padded 50048 cells
NPAD = NT * 128             # 50048
TAB_ROWS = NPAD             # table rows
HALF = 25024                # int16-index table split
BPH = 10                    # blocks (128 edges) per half per window
BPW = 2 * BPH               # 20 blocks per window
SLOTS_H = BPH * 128         # 1280 slots per half
SLOTS_W = BPW * 128         # 2560 slots per window
RCOL = 256                  # table row cols (bf16) = 512B
XCOL = 128                  # xm cols
TRACE = False
NW_RUN = int(os.environ.get("KERNEL_NW", NW))
SIM_SAFE = os.environ.get("KERNEL_SIM_SAFE", "0") == "1"
STAGE = int(os.environ.get("KERNEL_STAGE", "3"))

_CACHED = {}


def _build_nc():
    nc = bacc.Bacc(None)

    # ---- per-core inputs ----
    x_bf = nc.declare_dram_parameter("x_bf", [NPAD, C_IN], BF16, isOutput=False)
    x_own = nc.declare_dram_parameter("x_own", [CPC, C_IN], BF16, isOutput=False)
    w_all = nc.declare_dram_parameter("w_all", [C_IN, 264], BF16, isOutput=False)
    w_own = nc.declare_dram_parameter("w_own", [C_IN, 136], BF16, isOutput=False)
    b_rep = nc.declare_dram_parameter("b_rep", [128, 128], F32, isOutput=False)
    iota_in = nc.declare_dram_parameter("iota", [128, 128], BF16, isOutput=False)
    ident_in = nc.declare_dram_parameter("ident", [128, 128], BF16, isOutput=False)
    idx16 = [
        nc.declare_dram_parameter(f"idx16_{s}", [128, NW * 2 * (SLOTS_H // 16)], I16,
                                  isOutput=False)
        for s in range(2)
    ]
    tgtl = [
        nc.declare_dram_parameter(f"tgtl_{s}", [128, NW * BPW], F32, isOutput=False)
        for s in range(2)
    ]
    cnts = [
        nc.declare_dram_parameter(f"cnt_{s}", [1, NW * 2], mybir.dt.int32,
                                  isOutput=False)
        for s in range(2)
    ]
    out = nc.declare_dram_parameter("out", [CPC, HD], F32, isOutput=True)

    # ---- DRAM internals ----
    tables = [nc.dram_tensor(f"table_{s}", [TAB_ROWS, RCOL], BF16) for s in range(2)]

    IPH = SLOTS_H // 16      # idx16 cols per half (80)

    with tile.TileContext(nc) as tc:
        # ---------- persistent SBUF ----------
        with tc.tile_pool(name="persist", bufs=1) as pers:
            t_iota = pers.tile([128, 128], BF16)
            t_ident = pers.tile([128, 128], BF16)
            t_brep = pers.tile([128, 128], F32)
            t_idx = [pers.tile([128, NW * 2 * IPH], I16, tag=f"idx{s}", name=f"tidx{s}") for s in range(2)]
            t_tgtl = [pers.tile([128, NW * BPW], F32, tag=f"tgtl{s}", name=f"ttgtl{s}") for s in range(2)]
            t_sdw = [pers.tile([128, NW * 2 * HEADS], BF16, tag=f"sdw{s}", name=f"tsdw{s}") for s in range(2)]
            t_skip = pers.tile([128, NW * 128], F32)
            t_cnt = [pers.tile([1, NW * 2], mybir.dt.int32, tag=f"cnt{s}",
                               name=f"tcnt{s}") for s in range(2)]

            nc.sync.dma_start(out=t_iota[:], in_=iota_in[:])
            nc.sync.dma_start(out=t_ident[:], in_=ident_in[:])
            nc.sync.dma_start(out=t_brep[:], in_=b_rep[:])
            for s in range(2):
                nc.sync.dma_start(out=t_idx[s][:], in_=idx16[s][:])
                nc.sync.dma_start(out=t_tgtl[s][:], in_=tgtl[s][:])
                nc.sync.dma_start(out=t_cnt[s][:], in_=cnts[s][:])

            # ---------- node phase ----------
            with tc.tile_pool(name="node_sb", bufs=1) as nsb, \
                 tc.tile_pool(name="node_stage", bufs=3) as nst, \
                 tc.tile_pool(name="node_ps", bufs=4, space="PSUM") as nps:
                t_wall = nsb.tile([128, 264], BF16)
                t_wown = nsb.tile([128, 136], BF16)
                nc.sync.dma_start(out=t_wall[:], in_=w_all[:])
                nc.sync.dma_start(out=t_wown[:], in_=w_own[:])

                t_xT = nsb.tile([128, NPAD], BF16)
                CH = 3072  # transpose-dma chunk (rows, multiple of 128)
                for c0 in range(0, NPAD, CH):
                    ce = min(CH, NPAD - c0)
                    nc.sync.dma_start(out=t_xT[:, c0:c0 + ce],
                                      in_=x_bf[c0:c0 + ce, :], transpose=True)

                for t in range(NT):
                    ps = nps.tile([128, 264], F32, tag="nps")
                    nc.tensor.matmul(ps[:], t_xT[:, t * 128:(t + 1) * 128],
                                     t_wall[:], start=True, stop=True)
                    for s in range(2):
                        stg = nst.tile([128, RCOL], BF16, tag=f"stg{s}", name=f"stg{s}")
                        if SIM_SAFE or t < 3:
                            nc.gpsimd.memset(stg[:], 0)
                        if s == 0:
                            nc.vector.tensor_copy(out=stg[:, 0:XCOL],
                                                  in_=ps[:, 0:128])
                        else:
                            nc.scalar.copy(out=stg[:, 0:XCOL],
                                           in_=ps[:, 128:256])
                        ss_view = stg[:, XCOL:XCOL + 8].bitcast(F32)
                        nc.vector.tensor_copy(out=ss_view,
                                              in_=ps[:, 256 + 4 * s:256 + 4 * s + 4])
                        nc.sync.dma_start(out=tables[s][t * 128:(t + 1) * 128, :],
                                          in_=stg[:])

                # own pass: sd + skip for this core's cells
                t_xoT = nsb.tile([128, CPC], BF16)
                for c0 in range(0, CPC, CH):
                    ce = min(CH, CPC - c0)
                    nc.sync.dma_start(out=t_xoT[:, c0:c0 + ce],
                                      in_=x_own[c0:c0 + ce, :], transpose=True)
                for t in range(NW):
                    ps = nps.tile([128, 136], F32, tag="ops")
                    nc.tensor.matmul(ps[:], t_xoT[:, t * 128:(t + 1) * 128],
                                     t_wown[:], start=True, stop=True)
                    for s in range(2):
                        hi = t_sdw[s][:, t * 2 * HEADS:t * 2 * HEADS + HEADS]
                        lo = t_sdw[s][:, t * 2 * HEADS + HEADS:(t + 1) * 2 * HEADS]
                        nc.vector.tensor_copy(out=hi, in_=ps[:, 4 * s:4 * s + 4])
                        nc.vector.tensor_tensor(out=lo, in0=ps[:, 4 * s:4 * s + 4],
                                                in1=hi,
                                                op=mybir.AluOpType.subtract)
                    # skip with bias
                    nc.vector.scalar_tensor_tensor(
                        out=t_skip[:, t * 128:(t + 1) * 128],
                        in0=ps[:, 8:136], scalar=0.0,
                        in1=t_brep[:],
                        op0=mybir.AluOpType.add, op1=mybir.AluOpType.add)

            # ---------- edge phase ----------
            with tc.tile_pool(name="eg", bufs=2) as egp, \
                 tc.tile_pool(name="ea", bufs=2) as eap, \
                 tc.tile_pool(name="esm", bufs=2) as esm, \
                 tc.tile_pool(name="eat", bufs=4) as eat, \
                 tc.tile_pool(name="eps", bufs=2, space="PSUM") as epp, \
                 tc.tile_pool(name="epsb", bufs=2, space="PSUM") as epb, \
                 tc.tile_pool(name="ecmb", bufs=2) as ecmb:
                for w in range(NW_RUN):
                    psA = [None, None]
                    for s in range(2 if STAGE >= 1 else 0):
                        G = egp.tile([128, BPW, RCOL], BF16, tag="G")
                        if SIM_SAFE or w == 0:
                            nc.gpsimd.memset(G[:], 0)
                        for half in range(2):
                            nreg = nc.gpsimd.value_load(
                                t_cnt[s][0:1, w * 2 + half:w * 2 + half + 1])
                            nc.gpsimd.dma_gather(
                                out_ap=G[:, half * BPH:(half + 1) * BPH, :],
                                in_ap=tables[s][half * HALF:half * HALF + HALF, :],
                                idxs_ap=t_idx[s][:, (w * 2 + half) * IPH:
                                                 (w * 2 + half + 1) * IPH],
                                num_idxs=SLOTS_H,
                                num_idxs_reg=nreg,
                                elem_size=RCOL,
                                single_packet=False,
                            )
                        if STAGE < 2:
                            continue
                        A = eap.tile([128, BPW, 128], BF16, tag="A")
                        sd_ps = epb.tile([128, BPW * 2 * HEADS], F32, tag="sdps")
                        for b in range(BPW):
                            nc.vector.tensor_scalar(
                                out=A[:, b, :], in0=t_iota[:],
                                scalar1=t_tgtl[s][:, w * BPW + b:w * BPW + b + 1],
                                scalar2=None, op0=mybir.AluOpType.is_equal)
                        for b in range(BPW):
                            atp = epb.tile([128, 128], BF16, tag="atp")
                            nc.tensor.transpose(out=atp[:], in_=A[:, b, :],
                                                identity=t_ident[:])
                            at_sb = eat.tile([128, 128], BF16, tag="atsb")
                            nc.vector.tensor_copy(out=at_sb[:], in_=atp[:])
                            nc.tensor.matmul(
                                sd_ps[:, b * 2 * HEADS:(b + 1) * 2 * HEADS],
                                at_sb[:],
                                t_sdw[s][:, w * 2 * HEADS:(w + 1) * 2 * HEADS],
                                start=True, stop=True)
                        # window-batched softmax weights
                        alpha = esm.tile([128, BPW * HEADS], F32, tag="alpha")
                        sd3 = sd_ps[:].rearrange("p (b two h) -> p b two h", two=2,
                                                 h=HEADS)
                        nc.vector.tensor_tensor(
                            out=alpha[:].rearrange("p (b h) -> p b h", h=HEADS),
                            in0=G[:, :, XCOL:XCOL + 8].bitcast(F32),
                            in1=sd3[:, :, 0, :], op=mybir.AluOpType.add)
                        nc.vector.tensor_tensor(
                            out=alpha[:].rearrange("p (b h) -> p b h", h=HEADS),
                            in0=alpha[:].rearrange("p (b h) -> p b h", h=HEADS),
                            in1=sd3[:, :, 1, :], op=mybir.AluOpType.add)
                        lr = esm.tile([128, BPW * HEADS], F32, tag="lr")
                        nc.vector.scalar_tensor_tensor(
                            out=lr[:], in0=alpha[:], scalar=NEG_SLOPE,
                            in1=alpha[:],
                            op0=mybir.AluOpType.mult, op1=mybir.AluOpType.max)
                        e_w = esm.tile([128, BPW * HEADS], F32, tag="ew")
                        nc.scalar.activation(out=e_w[:], in_=lr[:],
                                             func=mybir.ActivationFunctionType.Exp)
                        if STAGE < 3:
                            continue
                        pme = egp.tile([128, BPW, 132], BF16, tag="pme")
                        nc.vector.tensor_copy(
                            out=pme[:, :, 128:132],
                            in_=e_w[:].rearrange("p (b h) -> p b h", h=HEADS))
                        ps_agg = epp.tile([128, 132], F32, tag=f"agg{s}")
                        for b in range(BPW):
                            ew_b = e_w[:, b * HEADS:(b + 1) * HEADS]
                            ew_bc = bass.AP(ew_b.tensor, ew_b.offset,
                                            [ew_b.ap[0], [1, HEADS], [0, D_OUT]])
                            nc.vector.tensor_tensor(
                                out=pme[:, b, 0:XCOL].rearrange(
                                    "p (h d) -> p h d", h=HEADS),
                                in0=G[:, b, 0:XCOL].rearrange(
                                    "p (h d) -> p h d", h=HEADS),
                                in1=ew_bc,
                                op=mybir.AluOpType.mult)
                            nc.tensor.matmul(ps_agg[:], A[:, b, :], pme[:, b, :],
                                             start=(b == 0), stop=(b == BPW - 1))
                        psA[s] = ps_agg

                    # ---- combine window ----
                    if STAGE < 3:
                        outt0 = ecmb.tile([128, 128], F32, tag="outt")
                        nc.vector.tensor_scalar_max(
                            outt0[:], t_skip[:, w * 128:(w + 1) * 128], 0.0)
                        nc.sync.dma_start(out=out[w * 128:(w + 1) * 128, :],
                                          in_=outt0[:])
                        continue
                    rec = [None, None]
                    for s in range(2):
                        dn = ecmb.tile([128, HEADS], F32, tag=f"dn{s}")
                        nc.vector.tensor_scalar_add(dn[:], psA[s][:, 128:132], 1e-16)
                        rc = ecmb.tile([128, HEADS], F32, tag=f"rc{s}")
                        nc.vector.reciprocal(out=rc[:], in_=dn[:])
                        rec[s] = rc
                    acc = ecmb.tile([128, 128], F32, tag="acc")
                    r0 = rec[0][:]
                    r0b = bass.AP(r0.tensor, r0.offset,
                                  [r0.ap[0], [1, HEADS], [0, D_OUT]])
                    nc.vector.tensor_tensor(
                        out=acc[:].rearrange("p (h d) -> p h d", h=HEADS),
                        in0=psA[0][:, 0:128].rearrange("p (h d) -> p h d", h=HEADS),
                        in1=r0b, op=mybir.AluOpType.mult)
                    acc2 = ecmb.tile([128, 128], F32, tag="acc2")
                    r1 = rec[1][:]
                    r1b = bass.AP(r1.tensor, r1.offset,
                                  [r1.ap[0], [1, HEADS], [0, D_OUT]])
                    nc.vector.tensor_tensor(
                        out=acc2[:].rearrange("p (h d) -> p h d", h=HEADS),
                        in0=psA[1][:, 0:128].rearrange("p (h d) -> p h d", h=HEADS),
                        in1=r1b, op=mybir.AluOpType.mult)
                    nc.vector.tensor_add(out=acc[:], in0=acc[:], in1=acc2[:])
                    nc.vector.tensor_add(out=acc[:], in0=acc[:],
                                         in1=t_skip[:, w * 128:(w + 1) * 128])
                    outt = ecmb.tile([128, 128], F32, tag="outt")
                    nc.vector.tensor_scalar_max(outt[:], acc[:], 0.0)
                    nc.sync.dma_start(out=out[w * 128:(w + 1) * 128, :], in_=outt[:])

    nc.finalize()
    return nc


def _fold(W, a):
    # W: [C_IN, HD] f32, a: [HEADS, D_OUT] -> [C_IN, HEADS]
    return np.einsum("chd,hd->ch",
                     W.astype(np.float64).reshape(C_IN, HEADS, D_OUT),
                     a.astype(np.float64)).astype(np.float32)


def _edge_arrays(tgt, src):
    """Per-core idx16 / tgtl / count arrays for one edge set."""
    idx_all = np.full((N_CORES, 128, NW * 2 * (SLOTS_H // 16)), -1, np.int16)
    tgl_all = np.full((N_CORES, 128, NW * BPW), -1.0, np.float32)
    cnt_all = np.zeros((N_CORES, 1, NW * 2), np.int32)
    order = np.argsort(tgt, kind="stable")
    tgt_s = tgt[order]
    src_s = src[order]
    core_of = tgt_s // CPC
    core_of = np.minimum(core_of, N_CORES - 1)
    for c in range(N_CORES):
        m = core_of == c
        tc_, sc_ = tgt_s[m] - c * CPC, src_s[m]
        wi = tc_ // 128
        tl = tc_ - wi * 128
        for w in range(NW):
            mw = wi == w
            tw, sw = tl[mw], sc_[mw]
            for half in range(2):
                if half == 0:
                    mh = sw < HALF
                    sidx = sw[mh]
                else:
                    mh = sw >= HALF
                    sidx = sw[mh] - HALF
                th = tw[mh]
                n = len(sidx)
                if n > SLOTS_H:
                    raise OverflowError("half-window overflow")
                flat_i = np.full(SLOTS_H, -1, np.int16)
                flat_i[:n] = sidx.astype(np.int16)
                wrap = flat_i.reshape(SLOTS_H // 16, 16).T  # [16, IPH]
                col0 = (w * 2 + half) * (SLOTS_H // 16)
                idx_all[c, :, col0:col0 + SLOTS_H // 16] = np.tile(wrap, (8, 1))
                # tgtl: slot (b,p): block b within window = half*BPH + i//128
                tl_flat = np.full(SLOTS_H, -1.0, np.float32)
                tl_flat[:n] = th.astype(np.float32)
                blk = tl_flat.reshape(BPH, 128)  # [b, p]
                b0 = w * BPW + half * BPH
                tgl_all[c, :, b0:b0 + BPH] = blk.T
                cnt_all[c, 0, w * 2 + half] = n
    return idx_all, tgl_all, cnt_all


def kernel(x, lower_tgt, lower_src, upper_tgt, upper_src,
           W_low, a_src_low, a_dst_low, W_up, a_src_up, a_dst_up,
           W_skip, b_skip):
    if "nc" not in _CACHED:
        _CACHED["nc"] = _build_nc()
    nc = _CACHED["nc"]

    x = np.asarray(x, np.float32)
    x_bf_full = np.zeros((NPAD, C_IN), ml_dtypes.bfloat16)
    x_bf_full[:N_CELLS] = x.astype(ml_dtypes.bfloat16)

    w_all = np.zeros((C_IN, 264), np.float32)
    w_all[:, 0:128] = W_low
    w_all[:, 128:256] = W_up
    w_all[:, 256:260] = _fold(W_low, a_src_low)
    w_all[:, 260:264] = _fold(W_up, a_src_up)
    w_all = w_all.astype(ml_dtypes.bfloat16)

    w_own = np.zeros((C_IN, 136), np.float32)
    w_own[:, 0:4] = _fold(W_low, a_dst_low)
    w_own[:, 4:8] = _fold(W_up, a_dst_up)
    w_own[:, 8:136] = EPS * W_skip
    w_own = w_own.astype(ml_dtypes.bfloat16)

    b_rep = np.broadcast_to((EPS * b_skip).astype(np.float32), (128, 128)).copy()
    iota = np.broadcast_to(np.arange(128, dtype=ml_dtypes.bfloat16),
                           (128, 128)).copy()
    ident = np.eye(128, dtype=ml_dtypes.bfloat16)

    idx0, tgl0, cnt0 = _edge_arrays(np.asarray(lower_tgt), np.asarray(lower_src))
    idx1, tgl1, cnt1 = _edge_arrays(np.asarray(upper_tgt), np.asarray(upper_src))

    in_maps = []
    for c in range(N_CORES):
        xo = np.zeros((CPC, C_IN), ml_dtypes.bfloat16)
        lo, hi = c * CPC, min((c + 1) * CPC, N_CELLS)
        if c == N_CORES - 1:
            hi = N_CELLS
        xo[:hi - lo] = x[lo:hi].astype(ml_dtypes.bfloat16)
        in_maps.append(dict(
            x_bf=x_bf_full, x_own=xo, w_all=w_all, w_own=w_own, b_rep=b_rep,
            iota=iota, ident=ident,
            idx16_0=idx0[c], idx16_1=idx1[c], tgtl_0=tgl0[c], tgtl_1=tgl1[c],
            cnt_0=cnt0[c], cnt_1=cnt1[c],
        ))

    res = run_bass_kernel_spmd(nc, in_maps, core_ids=list(range(N_CORES)),
                               trace=TRACE)
    outs = []
    for c in range(N_CORES):
        lo = c * CPC
        hi = min(lo + CPC, N_CELLS)
        outs.append(res.results[c]["out"][:hi - lo])
    full = np.concatenate(outs, axis=0)
    if TRACE:
        kernel.last_exec_ns = res.exec_time_ns
        kernel.last_results = res
    return full.astype(np.float32)

